# revision 9
# baseline (speedup 1.0000x reference)
"""Trainium2 Bass kernel for nn_C2PSA_FMFFN (C2PSA with frequency-modulated FFN).

Sharding: data-parallel over batch, B=32 -> 4 images per core on 8 cores.

Per-core layout: activations as [channels(partitions), pixels(free)], pixels in
window-major order (img, wy, wx, py, px).  Restructurings:
 - FMFFN's rfft2 -> complex modulation -> irfft2 == per-channel 4x4 circular
   convolution within each 4x4 window == per-channel 16x16 matrix; executed on
   the tensor engine as 85 block-diagonal [128x128] matmuls (8 channels x 16
   window-pixels) with SBUF<->SBUF shuffle DMAs around it.
 - Attention computed as S^T = k^T q (4 heads concurrently via PE row tiling),
   softmax without max-subtraction (scores are small; exp stays in fp32 range),
   denominators from an appended ones-column on v^T inside the PV matmul.
 - Depthwise 3x3 conv as 9 per-partition-scalar multiply-accumulate DVE ops
   decomposed into affine pieces of the window-major layout.
GEMMs in bf16 with fp32 PSUM accumulation; softmax/normalization in fp32.
"""
import numpy as np
import ml_dtypes
from contextlib import ExitStack

import concourse.bass as bass
import concourse.tile as tile
from concourse import bacc, mybir
from concourse import bass_utils

F32, BF16 = mybir.dt.float32, mybir.dt.bfloat16
AF = mybir.ActivationFunctionType
ALU = mybir.AluOpType

B, C1, C, NH, KD, HD, HID, NB, WS = 32, 512, 256, 4, 32, 64, 680, 3, 4
SCALE = KD ** -0.5
NCORE = 8
BPC = B // NCORE              # images per core = 4
NPIX = BPC * 1024             # 4096
NWIN = BPC * 64               # 256 windows per core
NG = HID // 8                 # 85 window channel-groups
NKT2 = (HID + 127) // 128     # 6 k-tiles over HID

bf16 = ml_dtypes.bfloat16


# ------------------------------------------------------------------ host prep
def _win_kernels(cw_i):
    """cw_i: [4, 3, HID, 2] -> real circular conv kernels [HID, 4, 4]."""
    wc = cw_i[..., 0] + 1j * cw_i[..., 1]
    delta = np.zeros((4, 4, 1))
    delta[0, 0, 0] = 1.0
    f = np.fft.rfft2(delta, axes=(0, 1), norm='ortho')
    h = np.fft.irfft2(f * wc, s=(4, 4), axes=(0, 1), norm='ortho')
    return np.transpose(h, (2, 0, 1))


def _win_blockdiag(cw):
    """cw: [NB,4,3,HID,2] -> [NB,NG,128,128] block-diag lhsT (bf16).
    lhsT[(c8,q),(c8,p)] = M_c[p,q];  M_c[p,q] = h_c[(py-qy)%4,(px-qx)%4]."""
    out = np.zeros((NB, NG, 128, 128), np.float32)
    for i in range(NB):
        h = _win_kernels(cw[i])                       # [HID,4,4]
        M = np.zeros((HID, 16, 16), np.float32)
        for pp in range(16):
            ppy, ppx = divmod(pp, 4)
            for qq in range(16):
                qqy, qqx = divmod(qq, 4)
                M[:, pp, qq] = h[:, (ppy - qqy) % 4, (ppx - qqx) % 4]
        for g in range(NG):
            for c8 in range(8):
                out[i, g, c8 * 16:(c8 + 1) * 16, c8 * 16:(c8 + 1) * 16] = M[g * 8 + c8].T
    return out.astype(bf16)


def _prep_weights(inp):
    qkv = np.asarray(inp['qkv_w'], np.float32).reshape(NB, NH, 128, C)
    wq = qkv[:, :, :32].reshape(NB, NH * 32, C)
    wk = qkv[:, :, 32:64].reshape(NB, NH * 32, C)
    wv = qkv[:, :, 64:].reshape(NB, NH * 64, C)
    return {
        'cv1_wT': np.asarray(inp['cv1_w'], np.float32).T.astype(bf16).copy(),
        'cv2_wT': np.asarray(inp['cv2_w'], np.float32).T.astype(bf16).copy(),
        'qk_wT': np.concatenate([wq, wk], 1).transpose(0, 2, 1).astype(bf16).copy(),
        'v_wT': wv.transpose(0, 2, 1).astype(bf16).copy(),
        'proj_wT': np.asarray(inp['proj_w'], np.float32).transpose(0, 2, 1).astype(bf16).copy(),
        'pe_w': np.asarray(inp['pe_w'], np.float32).reshape(NB, C, 9).copy(),
        'fc1_wT': np.asarray(inp['fc1_w'], np.float32).transpose(0, 2, 1).astype(bf16).copy(),
        'fc2_wT': np.asarray(inp['fc2_w'], np.float32).transpose(0, 2, 1).astype(bf16).copy(),
        'wmb': _win_blockdiag(np.asarray(inp['cw'], np.float64)),
    }


def _dpieces(dd):
    """(d0, nd, pd0, npd, sw, sp): dst (w, p) ranges + src offsets for shift dd."""
    if dd == 0:
        return [(0, 8, 0, 4, 0, 0)]
    if dd == 1:
        return [(0, 8, 0, 3, 0, 1), (0, 7, 3, 1, 1, -3)]
    return [(0, 8, 1, 3, 0, -1), (1, 7, 0, 1, -1, 3)]


# ------------------------------------------------------------------ emit
def _emit(ctx, nc, tc, d):
    sb = ctx.enter_context(tc.tile_pool(name="sb", bufs=1))
    ps = ctx.enter_context(tc.tile_pool(name="ps", bufs=8, space="PSUM"))
    wpool = ctx.enter_context(tc.tile_pool(name="wpool", bufs=1))
    big = ctx.enter_context(tc.tile_pool(name="big", bufs=1))
    tp = ctx.enter_context(tc.tile_pool(name="tp", bufs=2))
    tp3 = ctx.enter_context(tc.tile_pool(name="tp3", bufs=3))
    tp4 = ctx.enter_context(tc.tile_pool(name="tp4", bufs=4))

    def pst(name):
        return ps.tile([128, 512], F32, tag="ps", name=name)

    def load_w(name, dram, ktiles, mcols, dtype=BF16):
        t = wpool.tile([128, ktiles * mcols], dtype, tag=name, name=name)
        nc.sync.dma_start(t[:].rearrange("p (k m) -> p k m", k=ktiles),
                          dram.rearrange("(k p) m -> p k m", p=128))
        return t

    # ---------------- static weights ----------------
    cv1w = load_w("cv1w", d['cv1_wT'][:], 4, 512)
    cv2w = load_w("cv2w", d['cv2_wT'][:], 4, 512)

    # ---------------- input -> xb bf16 window-major ----------------
    xb = big.tile([128, 4 * NPIX], BF16, tag="big", name="xb")
    for kt in range(4):
        for img in range(BPC):
            for hh in range(2):
                xc = tp.tile([128, 512], F32, tag="xc", name=f"xc{kt}_{img}_{hh}")
                nc.sync.dma_start(
                    xc[:], d['x'][kt * 128:(kt + 1) * 128,
                                  img * 1024 + hh * 512: img * 1024 + (hh + 1) * 512])
                src = xc[:].rearrange("p (wy py wx px) -> p wy py wx px",
                                      wy=4, py=4, wx=8, px=4).transpose([0, 1, 3, 2, 4])
                dst = xb[:, kt * NPIX + img * 1024 + hh * 512:
                         kt * NPIX + img * 1024 + (hh + 1) * 512]
                dst = dst.rearrange("p (wy wx py px) -> p wy wx py px",
                                    wy=4, wx=8, py=4, px=4)
                nc.vector.tensor_copy(dst, src)

    # ---------------- cv1 + silu ----------------
    a_sb = sb.tile([128, 2 * NPIX], BF16, name="a_sb")
    bfb = sb.tile([128, 2 * NPIX], BF16, name="bfb")
    for mt in range(4):
        for img in range(BPC):
            for nch in range(2):
                p = pst(f"cv1p_{mt}_{img}_{nch}")
                col = img * 1024 + nch * 512
                for kt in range(4):
                    nc.tensor.matmul(
                        p[:], cv1w[:, kt * 512 + mt * 128: kt * 512 + (mt + 1) * 128],
                        xb[:, kt * NPIX + col: kt * NPIX + col + 512],
                        start=(kt == 0), stop=(kt == 3))
                dst = (a_sb if mt < 2 else bfb)
                mm = mt % 2
                nc.scalar.activation(dst[:, mm * NPIX + col: mm * NPIX + col + 512],
                                     p[:], AF.Silu)

    # ---------------- blocks ----------------
    bfm = sb.tile([128, 2 * NPIX], BF16, name="bfm")       # (wp, img, win) order
    for blk in range(NB):
        qkw = load_w(f"qkw", d['qk_wT'][blk], 2, 256)
        vw = load_w(f"vw", d['v_wT'][blk], 2, 256)
        projw = load_w(f"projw", d['proj_wT'][blk], 2, 256)
        pew = load_w(f"pew", d['pe_w'][blk], 2, 9, dtype=F32)
        fc1w = load_w(f"fc1w", d['fc1_wT'][blk], 2, HID)
        fc2w = wpool.tile([128, NKT2 * 256], BF16, tag="fc2w", name=f"fc2w{blk}")
        nc.sync.dma_start(
            fc2w[:].rearrange("p (k m) -> p k m", k=NKT2)[:, 0:5],
            d['fc2_wT'][blk, 0:640].rearrange("(k p) m -> p k m", p=128))
        nc.sync.dma_start(fc2w[0:40, 5 * 256:6 * 256], d['fc2_wT'][blk, 640:680, :])

        # ---- attention, per image ----
        for img in range(BPC):
            icol = img * 1024
            # qk GEMM -> qk_img [128, 2048] (cols: q 0-1023, k 1024-2047)
            qk_img = tp.tile([128, 2048], BF16, tag="qk", name=f"qk{blk}_{img}")
            for mt in range(2):
                for nch in range(2):
                    p = pst(f"qkp{blk}_{img}_{mt}_{nch}")
                    for kt in range(2):
                        nc.tensor.matmul(
                            p[:], qkw[:, kt * 256 + mt * 128: kt * 256 + (mt + 1) * 128],
                            bfb[:, kt * NPIX + icol + nch * 512: kt * NPIX + icol + nch * 512 + 512],
                            start=(kt == 0), stop=(kt == 1))
                    nc.vector.tensor_copy(
                        qk_img[:, mt * 1024 + nch * 512: mt * 1024 + nch * 512 + 512], p[:])

            # vT GEMM -> vt [128, 8*260] (j-tile major; cols h*65+d, col 64 = ones)
            vt = tp.tile([128, 8 * 260], BF16, tag="vt", name=f"vt{blk}_{img}")
            for jt in range(8):
                p = pst(f"vtp{blk}_{img}_{jt}")
                for kt in range(2):
                    nc.tensor.matmul(
                        p[:, 0:256],
                        bfb[:, kt * NPIX + icol + jt * 128: kt * NPIX + icol + (jt + 1) * 128],
                        vw[:, kt * 256:(kt + 1) * 256],
                        start=(kt == 0), stop=(kt == 1))
                dst = vt[:, jt * 260:(jt + 1) * 260].rearrange("p (h e) -> p h e", h=4)
                nc.vector.tensor_copy(dst[:, :, 0:64],
                                      p[:, 0:256].rearrange("p (h e) -> p h e", h=4))
            ones_ap = vt[:].rearrange("p (j h e) -> p j h e", j=8, h=4)[:, :, :, 64:65]
            nc.vector.memset(ones_ap, 1.0)

            # v GEMM -> v_sb [256ch, 1024] bf16 in RASTER order (dwconv input)
            v_sb = tp.tile([128, 2 * 1024], BF16, tag="v_sb", name=f"v{blk}_{img}")
            for ct in range(2):
                for nch in range(2):
                    p = pst(f"vp{blk}_{img}_{ct}_{nch}")
                    for kt in range(2):
                        nc.tensor.matmul(
                            p[:], vw[:, kt * 256 + ct * 128: kt * 256 + (ct + 1) * 128],
                            bfb[:, kt * NPIX + icol + nch * 512: kt * NPIX + icol + nch * 512 + 512],
                            start=(kt == 0), stop=(kt == 1))
                    # psum cols (wy4, wx8, py4, px4) wm -> raster dst (4D copy)
                    dst = v_sb[:, ct * 1024 + nch * 512: ct * 1024 + nch * 512 + 512]
                    dst = dst.rearrange("p (wy py wx px) -> p wy wx py px",
                                        wy=4, py=4, wx=8, px=4)
                    nc.vector.tensor_copy(dst, p[:])

            # S^T + PV per i-chunk
            attn_f = tp.tile([128, 2048], F32, tag="scr8", name=f"at{blk}_{img}")
            for ich in range(2):
                pvs = [ps.tile([128, 512], F32, tag="ps", name=f"pv{blk}_{img}_{ich}_{h}")
                       for h in range(NH)]
                for jt in range(8):
                    sps = [ps.tile([128, 512], F32, tag="ps",
                                   name=f"s{blk}_{img}_{ich}_{jt}_{h}") for h in range(NH)]
                    for h in range(NH):
                        nc.tensor.matmul(
                            sps[h][:],
                            qk_img[32 * h:32 * h + 32, 1024 + jt * 128: 1024 + (jt + 1) * 128],
                            qk_img[32 * h:32 * h + 32, ich * 512: ich * 512 + 512],
                            start=True, stop=True, tile_position=(32 * h, 0))
                    pb = [tp4.tile([128, 512], BF16, tag="pb",
                                   name=f"p{blk}_{img}_{ich}_{jt}_{h}") for h in range(NH)]
                    for h in range(NH):
                        nc.scalar.activation(pb[h][:], sps[h][:], AF.Exp, scale=SCALE)
                    for h in range(NH):
                        nc.tensor.matmul(
                            pvs[h][0:65, :],
                            vt[:, jt * 260 + h * 65: jt * 260 + (h + 1) * 65],
                            pb[h][:], start=(jt == 0), stop=(jt == 7))
                for h in range(NH):
                    rs = tp.tile([1, 512], F32, tag="rs", name=f"rs{blk}_{img}_{ich}_{h}")
                    nc.scalar.copy(rs[:], pvs[h][64:65, :])
                    nc.vector.reciprocal(rs[:], rs[:])
                    rb = tp.tile([64, 512], F32, tag="rb", name=f"rb{blk}_{img}_{ich}_{h}")
                    nc.sync.dma_start(
                        rb[:], rs[:].unsqueeze(1).broadcast_to([1, 64, 512]))
                    ct, r0 = divmod(h * 64, 128)
                    nc.vector.tensor_tensor(
                        attn_f[r0:r0 + 64, ct * 1024 + ich * 512: ct * 1024 + ich * 512 + 512],
                        pvs[h][0:64, :], rb[:], ALU.mult)

            # dwconv 3x3 on raster v_sb -> pe (raster, f32), then permute+add
            pe = tp.tile([128, 2048], F32, tag="scr8", name=f"pe{blk}_{img}")
            for ct in range(2):
                out2 = pe[:, ct * 1024:(ct + 1) * 1024].rearrange(
                    "p (y x) -> p y x", y=32)
                in2 = v_sb[:, ct * 1024:(ct + 1) * 1024].rearrange(
                    "p (y x) -> p y x", y=32)
                # center tap first: full-coverage init (pure multiply)
                nc.vector.tensor_scalar(out2, in2, pew[:, ct * 9 + 4: ct * 9 + 5],
                                        None, ALU.mult)
                for tap in range(9):
                    if tap == 4:
                        continue
                    dy, dx = tap // 3 - 1, tap % 3 - 1
                    y0, y1 = max(0, -dy), min(32, 32 - dy)
                    x0, x1 = max(0, -dx), min(32, 32 - dx)
                    dst = out2[:, y0:y1, x0:x1]
                    src = in2[:, y0 + dy:y1 + dy, x0 + dx:x1 + dx]
                    nc.vector.scalar_tensor_tensor(
                        dst, src, pew[:, ct * 9 + tap: ct * 9 + tap + 1],
                        dst, ALU.mult, ALU.add)
            # pe (raster) -> wm-ordered bf16, then add into attn_f
            for ct in range(2):
                pewm = tp.tile([128, 1024], BF16, tag="pewm", name=f"pw{blk}_{img}_{ct}")
                src = pe[:, ct * 1024:(ct + 1) * 1024].rearrange(
                    "p (wy py wx px) -> p wy wx py px", wy=8, py=4, wx=8, px=4)
                dst = pewm[:].rearrange(
                    "p (wy wx py px) -> p wy wx py px", wy=8, wx=8, py=4, px=4)
                nc.vector.tensor_copy(dst, src)
                acol = ct * 1024
                nc.vector.tensor_tensor(attn_f[:, acol:acol + 1024],
                                        attn_f[:, acol:acol + 1024], pewm[:], ALU.add)

            attn_b = tp.tile([128, 2048], BF16, tag="attn_b", name=f"ab{blk}_{img}")
            nc.vector.tensor_copy(attn_b[:], attn_f[:])
            # proj + residual add into bfb
            for mt in range(2):
                for nch in range(2):
                    p = pst(f"projp{blk}_{img}_{mt}_{nch}")
                    for kt in range(2):
                        nc.tensor.matmul(
                            p[:], projw[:, kt * 256 + mt * 128: kt * 256 + (mt + 1) * 128],
                            attn_b[:, kt * 1024 + nch * 512: kt * 1024 + nch * 512 + 512],
                            start=(kt == 0), stop=(kt == 1))
                    dst = bfb[:, mt * NPIX + icol + nch * 512: mt * NPIX + icol + nch * 512 + 512]
                    nc.vector.tensor_tensor(dst, p[:], dst, ALU.add)

        # ---- fmffn ----
        # bfm <- bfb permuted to (wp, img, win) order, per ct
        for ct in range(2):
            for img in range(BPC):
                src = bfb[:, ct * NPIX + img * 1024: ct * NPIX + (img + 1) * 1024]
                src = src.rearrange("p (wy wx py px) -> p py px (wy wx)",
                                    wy=8, wx=8, py=4, px=4)
                dst = bfm[:, ct * NPIX:(ct + 1) * NPIX].rearrange(
                    "p (py px iw) -> p py px iw", py=4, px=4)[:, :, :, img * 64:(img + 1) * 64]
                nc.vector.tensor_copy(dst, src)
        t2_sb = big.tile([128, NKT2 * NPIX], BF16, tag="big", name=f"t2_{blk}")
        for mt in range(NKT2):
            m0 = mt * 128
            mrows = min(128, HID - m0)
            ngr = (mrows + 7) // 8
            wmt = tp.tile([128, 16 * 128], BF16, tag="wmt", name=f"wmt{blk}_{mt}")
            nc.sync.dma_start(
                wmt[:, 0:ngr * 128].rearrange("p (g m) -> p g m", g=ngr),
                d['wmb'][blk, mt * 16: mt * 16 + ngr].rearrange("g p m -> p g m"))
            t1c = tp.tile([128, NPIX], BF16, tag="scr8", name=f"t1c{blk}_{mt}")
            for ch in range(8):
                p = pst(f"fc1p{blk}_{mt}_{ch}")
                for kt in range(2):
                    nc.tensor.matmul(
                        p[0:mrows, :],
                        fc1w[:, kt * HID + m0: kt * HID + m0 + mrows],
                        bfm[:, kt * NPIX + ch * 512: kt * NPIX + (ch + 1) * 512],
                        start=(kt == 0), stop=(kt == 1))
                nc.scalar.activation(t1c[0:mrows, ch * 512:(ch + 1) * 512],
                                     p[0:mrows, :], AF.Gelu)
            for gl in range(ngr):
                g = mt * 16 + gl
                t1s = tp3.tile([128, NWIN], BF16, tag="t1s", name=f"t1s{blk}_{g}")
                nc.sync.dma_start(t1s[:], t1c[gl * 8:(gl + 1) * 8, :])
                p = ps.tile([128, 256], F32, tag="ps", name=f"winp{blk}_{g}")
                nc.tensor.matmul(p[:], wmt[:, gl * 128:(gl + 1) * 128], t1s[:],
                                 start=True, stop=True)
                t2s = tp3.tile([128, NWIN], BF16, tag="t2s", name=f"t2s{blk}_{g}")
                nc.scalar.copy(t2s[:], p[:])
                kt2, r0 = divmod(g * 8, 128)
                nc.sync.dma_start(t2_sb[r0:r0 + 8, kt2 * NPIX:(kt2 + 1) * NPIX], t2s[:])
        # fc2 + residual
        for img in range(BPC):
            for mt in range(2):
                for nch in range(2):
                    p = pst(f"fc2p{blk}_{img}_{mt}_{nch}")
                    for kt in range(NKT2):
                        krows = min(128, HID - kt * 128)
                        rhs = t2_sb[0:krows, kt * NPIX:(kt + 1) * NPIX].rearrange(
                            "p (wp iw) -> p wp iw", wp=16)[:, :, img * 64:(img + 1) * 64]
                        rhs = rhs.transpose([0, 2, 1])[:, nch * 32:(nch + 1) * 32, :]
                        nc.tensor.matmul(
                            p[:], fc2w[0:krows, kt * 256 + mt * 128: kt * 256 + (mt + 1) * 128],
                            rhs, start=(kt == 0), stop=(kt == NKT2 - 1))
                    col = img * 1024 + nch * 512
                    dst = bfb[:, mt * NPIX + col: mt * NPIX + col + 512]
                    nc.vector.tensor_tensor(dst, p[:], dst, ALU.add)

    # ---------------- cv2 + silu -> y (raster) ----------------
    for mt in range(4):
        for img in range(BPC):
            for nch in range(2):
                p = pst(f"cv2p_{mt}_{img}_{nch}")
                col = img * 1024 + nch * 512
                for kt in range(4):
                    rhs = (a_sb if kt < 2 else bfb)[
                        :, (kt % 2) * NPIX + col: (kt % 2) * NPIX + col + 512]
                    nc.tensor.matmul(
                        p[:], cv2w[:, kt * 512 + mt * 128: kt * 512 + (mt + 1) * 128],
                        rhs, start=(kt == 0), stop=(kt == 3))
                yc = tp.tile([128, 512], F32, tag="yc", name=f"yc{mt}_{img}_{nch}")
                nc.scalar.activation(yc[:], p[:], AF.Silu)
                # yc cols (wy4, wx8, py4, px4) window-major -> raster in yr
                yr = tp.tile([128, 512], F32, tag="yr", name=f"yr{mt}_{img}_{nch}")
                dst = yr[:].rearrange("p (wy py wx px) -> p wy wx py px",
                                      wy=4, py=4, wx=8, px=4)
                nc.vector.tensor_copy(
                    dst, yc[:].rearrange("p (wy wx py px) -> p wy wx py px",
                                         wy=4, wx=8, py=4, px=4))
                nc.sync.dma_start(
                    d['y'][mt * 128:(mt + 1) * 128, col:col + 512], yr[:])


_BUILT = {}


def _build():
    if 'nc' in _BUILT:
        return _BUILT['nc']
    nc = bacc.Bacc("TRN2", target_bir_lowering=False, debug=False, num_devices=NCORE)
    d = {
        'x': nc.dram_tensor("x", [C1, NPIX], F32, kind="ExternalInput").ap(),
        'cv1_wT': nc.dram_tensor("cv1_wT", [C1, C1], BF16, kind="ExternalInput").ap(),
        'cv2_wT': nc.dram_tensor("cv2_wT", [C1, C1], BF16, kind="ExternalInput").ap(),
        'qk_wT': nc.dram_tensor("qk_wT", [NB, C, C], BF16, kind="ExternalInput").ap(),
        'v_wT': nc.dram_tensor("v_wT", [NB, C, C], BF16, kind="ExternalInput").ap(),
        'proj_wT': nc.dram_tensor("proj_wT", [NB, C, C], BF16, kind="ExternalInput").ap(),
        'pe_w': nc.dram_tensor("pe_w", [NB, C, 9], F32, kind="ExternalInput").ap(),
        'fc1_wT': nc.dram_tensor("fc1_wT", [NB, C, HID], BF16, kind="ExternalInput").ap(),
        'fc2_wT': nc.dram_tensor("fc2_wT", [NB, HID, C], BF16, kind="ExternalInput").ap(),
        'wmb': nc.dram_tensor("wmb", [NB, NG, 128, 128], BF16, kind="ExternalInput").ap(),
        'y': nc.dram_tensor("y", [C1, NPIX], F32, kind="ExternalOutput").ap(),
    }
    with tile.TileContext(nc) as tc:
        with ExitStack() as ctx:
            _emit(ctx, nc, tc, d)
    nc.compile()
    _BUILT['nc'] = nc
    return nc


def kernel(**inputs):
    nc = _build()
    w = _prep_weights(inputs)
    x = np.asarray(inputs['x'], np.float32)
    in_maps = []
    for core in range(NCORE):
        xs = np.ascontiguousarray(
            x[core * BPC:(core + 1) * BPC].transpose(1, 0, 2, 3).reshape(C1, NPIX))
        m = {'x': xs}
        m.update(w)
        in_maps.append(m)
    res = bass_utils.run_bass_kernel_spmd(nc, in_maps, core_ids=list(range(NCORE)))
    outs = []
    for core in range(NCORE):
        y = res.results[core]['y']                      # [512, NPIX] raster
        outs.append(y.reshape(C1, BPC, 32, 32).transpose(1, 0, 2, 3))
    return np.concatenate(outs, 0)


# revision 11
# speedup vs baseline: 1.0855x; 1.0855x over previous
"""Trainium2 Bass kernel for nn_C2PSA_FMFFN (C2PSA with frequency-modulated FFN).

Sharding: data-parallel over batch, B=32 -> 4 images per core on 8 cores.

Per-core layout: activations as [channels(partitions), pixels(free)], pixels in
window-major order (img, wy, wx, py, px).  Restructurings:
 - FMFFN's rfft2 -> complex modulation -> irfft2 == per-channel 4x4 circular
   convolution within each 4x4 window == per-channel 16x16 matrix; executed on
   the tensor engine as 85 block-diagonal [128x128] matmuls (8 channels x 16
   window-pixels) with SBUF<->SBUF shuffle DMAs around it.
 - Attention computed as S^T = k^T q (4 heads concurrently via PE row tiling),
   softmax without max-subtraction (scores are small; exp stays in fp32 range),
   denominators from an appended ones-column on v^T inside the PV matmul.
 - Depthwise 3x3 conv as 9 per-partition-scalar multiply-accumulate DVE ops
   decomposed into affine pieces of the window-major layout.
GEMMs in bf16 with fp32 PSUM accumulation; softmax/normalization in fp32.
"""
import numpy as np
import ml_dtypes
from contextlib import ExitStack

import concourse.bass as bass
import concourse.tile as tile
from concourse import bacc, mybir
from concourse import bass_utils

F32, BF16 = mybir.dt.float32, mybir.dt.bfloat16
AF = mybir.ActivationFunctionType
ALU = mybir.AluOpType

B, C1, C, NH, KD, HD, HID, NB, WS = 32, 512, 256, 4, 32, 64, 680, 3, 4
SCALE = KD ** -0.5
NCORE = 8
BPC = B // NCORE              # images per core = 4
NPIX = BPC * 1024             # 4096
NWIN = BPC * 64               # 256 windows per core
NG = HID // 8                 # 85 window channel-groups
NKT2 = (HID + 127) // 128     # 6 k-tiles over HID

bf16 = ml_dtypes.bfloat16


# ------------------------------------------------------------------ host prep
def _win_kernels(cw_i):
    """cw_i: [4, 3, HID, 2] -> real circular conv kernels [HID, 4, 4]."""
    wc = cw_i[..., 0] + 1j * cw_i[..., 1]
    delta = np.zeros((4, 4, 1))
    delta[0, 0, 0] = 1.0
    f = np.fft.rfft2(delta, axes=(0, 1), norm='ortho')
    h = np.fft.irfft2(f * wc, s=(4, 4), axes=(0, 1), norm='ortho')
    return np.transpose(h, (2, 0, 1))


def _win_blockdiag(cw):
    """cw: [NB,4,3,HID,2] -> [NB,NG,128,128] block-diag lhsT (bf16).
    lhsT[(c8,q),(c8,p)] = M_c[p,q];  M_c[p,q] = h_c[(py-qy)%4,(px-qx)%4]."""
    out = np.zeros((NB, NG, 128, 128), np.float32)
    for i in range(NB):
        h = _win_kernels(cw[i])                       # [HID,4,4]
        M = np.zeros((HID, 16, 16), np.float32)
        for pp in range(16):
            ppy, ppx = divmod(pp, 4)
            for qq in range(16):
                qqy, qqx = divmod(qq, 4)
                M[:, pp, qq] = h[:, (ppy - qqy) % 4, (ppx - qqx) % 4]
        for g in range(NG):
            for c8 in range(8):
                out[i, g, c8 * 16:(c8 + 1) * 16, c8 * 16:(c8 + 1) * 16] = M[g * 8 + c8].T
    return out.astype(bf16)


def _prep_weights(inp):
    qkv = np.asarray(inp['qkv_w'], np.float32).reshape(NB, NH, 128, C)
    wq = qkv[:, :, :32].reshape(NB, NH * 32, C)
    wk = qkv[:, :, 32:64].reshape(NB, NH * 32, C)
    wv = qkv[:, :, 64:].reshape(NB, NH * 64, C)
    return {
        'cv1_wT': np.asarray(inp['cv1_w'], np.float32).T.astype(bf16).copy(),
        'cv2_wT': np.asarray(inp['cv2_w'], np.float32).T.astype(bf16).copy(),
        'qk_wT': np.concatenate([wq, wk], 1).transpose(0, 2, 1).astype(bf16).copy(),
        'v_wT': wv.transpose(0, 2, 1).astype(bf16).copy(),
        'proj_wT': np.asarray(inp['proj_w'], np.float32).transpose(0, 2, 1).astype(bf16).copy(),
        'pe_w': np.asarray(inp['pe_w'], np.float32).reshape(NB, C, 9).copy(),
        'fc1_wT': np.asarray(inp['fc1_w'], np.float32).transpose(0, 2, 1).astype(bf16).copy(),
        'fc2_wT': np.asarray(inp['fc2_w'], np.float32).transpose(0, 2, 1).astype(bf16).copy(),
        'wmb': _win_blockdiag(np.asarray(inp['cw'], np.float64)),
    }


def _dpieces(dd):
    """(d0, nd, pd0, npd, sw, sp): dst (w, p) ranges + src offsets for shift dd."""
    if dd == 0:
        return [(0, 8, 0, 4, 0, 0)]
    if dd == 1:
        return [(0, 8, 0, 3, 0, 1), (0, 7, 3, 1, 1, -3)]
    return [(0, 8, 1, 3, 0, -1), (1, 7, 0, 1, -1, 3)]


# ------------------------------------------------------------------ emit
def _emit(ctx, nc, tc, d):
    sb = ctx.enter_context(tc.tile_pool(name="sb", bufs=1))
    ps = ctx.enter_context(tc.tile_pool(name="ps", bufs=8, space="PSUM"))
    wpool = ctx.enter_context(tc.tile_pool(name="wpool", bufs=1))
    big = ctx.enter_context(tc.tile_pool(name="big", bufs=1))
    tp = ctx.enter_context(tc.tile_pool(name="tp", bufs=2))
    tp3 = ctx.enter_context(tc.tile_pool(name="tp3", bufs=3))
    tp4 = ctx.enter_context(tc.tile_pool(name="tp4", bufs=4))

    def pst(name):
        return ps.tile([128, 512], F32, tag="ps", name=name)

    def load_w(name, dram, ktiles, mcols, dtype=BF16):
        t = wpool.tile([128, ktiles * mcols], dtype, tag=name, name=name)
        nc.sync.dma_start(t[:].rearrange("p (k m) -> p k m", k=ktiles),
                          dram.rearrange("(k p) m -> p k m", p=128))
        return t

    import os as _os
    for _rep in range(int(_os.environ.get("KREP", "1"))):
        _emit_body(nc, tc, d, pst, load_w, sb, ps, wpool, big, tp, tp3, tp4)


def _emit_body(nc, tc, d, pst, load_w, sb, ps, wpool, big, tp, tp3, tp4):
    # ---------------- static weights ----------------
    cv1w = load_w("cv1w", d['cv1_wT'][:], 4, 512)
    cv2w = load_w("cv2w", d['cv2_wT'][:], 4, 512)

    # ---------------- input -> xb bf16 window-major ----------------
    xb = big.tile([128, 4 * NPIX], BF16, tag="big", name="xb")
    for kt in range(4):
        for img in range(BPC):
            for hh in range(2):
                xc = tp.tile([128, 512], F32, tag="xc", name=f"xc{kt}_{img}_{hh}")
                nc.sync.dma_start(
                    xc[:], d['x'][kt * 128:(kt + 1) * 128,
                                  img * 1024 + hh * 512: img * 1024 + (hh + 1) * 512])
                src = xc[:].rearrange("p (wy py wx px) -> p wy py wx px",
                                      wy=4, py=4, wx=8, px=4).transpose([0, 1, 3, 2, 4])
                dst = xb[:, kt * NPIX + img * 1024 + hh * 512:
                         kt * NPIX + img * 1024 + (hh + 1) * 512]
                dst = dst.rearrange("p (wy wx py px) -> p wy wx py px",
                                    wy=4, wx=8, py=4, px=4)
                nc.vector.tensor_copy(dst, src)

    # ---------------- cv1 + silu ----------------
    a_sb = sb.tile([128, 2 * NPIX], BF16, name="a_sb")
    bfb = sb.tile([128, 2 * NPIX], BF16, name="bfb")
    for mt in range(4):
        for img in range(BPC):
            for nch in range(2):
                p = pst(f"cv1p_{mt}_{img}_{nch}")
                col = img * 1024 + nch * 512
                for kt in range(4):
                    nc.tensor.matmul(
                        p[:], cv1w[:, kt * 512 + mt * 128: kt * 512 + (mt + 1) * 128],
                        xb[:, kt * NPIX + col: kt * NPIX + col + 512],
                        start=(kt == 0), stop=(kt == 3))
                dst = (a_sb if mt < 2 else bfb)
                mm = mt % 2
                nc.scalar.activation(dst[:, mm * NPIX + col: mm * NPIX + col + 512],
                                     p[:], AF.Silu)

    # ---------------- blocks ----------------
    bfm = sb.tile([128, 2 * NPIX], BF16, name="bfm")       # (wp, img, win) order
    for blk in range(NB):
        qkw = load_w(f"qkw", d['qk_wT'][blk], 2, 256)
        vw = load_w(f"vw", d['v_wT'][blk], 2, 256)
        projw = load_w(f"projw", d['proj_wT'][blk], 2, 256)
        pew = load_w(f"pew", d['pe_w'][blk], 2, 9, dtype=F32)
        fc1w = load_w(f"fc1w", d['fc1_wT'][blk], 2, HID)
        fc2w = wpool.tile([128, NKT2 * 256], BF16, tag="fc2w", name=f"fc2w{blk}")
        nc.sync.dma_start(
            fc2w[:].rearrange("p (k m) -> p k m", k=NKT2)[:, 0:5],
            d['fc2_wT'][blk, 0:640].rearrange("(k p) m -> p k m", p=128))
        nc.sync.dma_start(fc2w[0:40, 5 * 256:6 * 256], d['fc2_wT'][blk, 640:680, :])

        # ---- attention, per image ----
        for img in range(BPC):
            icol = img * 1024
            # qk GEMM -> qk_img [128, 2048] (cols: q 0-1023, k 1024-2047)
            qk_img = tp.tile([128, 2048], BF16, tag="qk", name=f"qk{blk}_{img}")
            for mt in range(2):
                for nch in range(2):
                    p = pst(f"qkp{blk}_{img}_{mt}_{nch}")
                    for kt in range(2):
                        nc.tensor.matmul(
                            p[:], qkw[:, kt * 256 + mt * 128: kt * 256 + (mt + 1) * 128],
                            bfb[:, kt * NPIX + icol + nch * 512: kt * NPIX + icol + nch * 512 + 512],
                            start=(kt == 0), stop=(kt == 1))
                    nc.vector.tensor_copy(
                        qk_img[:, mt * 1024 + nch * 512: mt * 1024 + nch * 512 + 512], p[:])

            # vT GEMM -> vt [128, 8*260] (j-tile major; cols h*65+d, col 64 = ones)
            vt = tp.tile([128, 8 * 260], BF16, tag="vt", name=f"vt{blk}_{img}")
            for jt in range(8):
                p = pst(f"vtp{blk}_{img}_{jt}")
                for kt in range(2):
                    nc.tensor.matmul(
                        p[:, 0:256],
                        bfb[:, kt * NPIX + icol + jt * 128: kt * NPIX + icol + (jt + 1) * 128],
                        vw[:, kt * 256:(kt + 1) * 256],
                        start=(kt == 0), stop=(kt == 1))
                dst = vt[:, jt * 260:(jt + 1) * 260].rearrange("p (h e) -> p h e", h=4)
                nc.vector.tensor_copy(dst[:, :, 0:64],
                                      p[:, 0:256].rearrange("p (h e) -> p h e", h=4))
            ones_ap = vt[:].rearrange("p (j h e) -> p j h e", j=8, h=4)[:, :, :, 64:65]
            nc.vector.memset(ones_ap, 1.0)

            # v GEMM -> v_sb [256ch, 1024] bf16 in RASTER order (dwconv input)
            v_sb = tp.tile([128, 2 * 1024], BF16, tag="v_sb", name=f"v{blk}_{img}")
            for ct in range(2):
                for nch in range(2):
                    p = pst(f"vp{blk}_{img}_{ct}_{nch}")
                    for kt in range(2):
                        nc.tensor.matmul(
                            p[:], vw[:, kt * 256 + ct * 128: kt * 256 + (ct + 1) * 128],
                            bfb[:, kt * NPIX + icol + nch * 512: kt * NPIX + icol + nch * 512 + 512],
                            start=(kt == 0), stop=(kt == 1))
                    # psum cols (wy4, wx8, py4, px4) wm -> raster dst (4D copy)
                    dst = v_sb[:, ct * 1024 + nch * 512: ct * 1024 + nch * 512 + 512]
                    dst = dst.rearrange("p (wy py wx px) -> p wy wx py px",
                                        wy=4, py=4, wx=8, px=4)
                    nc.vector.tensor_copy(dst, p[:])

            # S^T + PV per i-chunk
            attn_f = tp.tile([128, 2048], F32, tag="scr8", name=f"at{blk}_{img}")
            for ich in range(2):
                pvs = [ps.tile([128, 512], F32, tag="ps", name=f"pv{blk}_{img}_{ich}_{h}")
                       for h in range(NH)]
                for jt in range(8):
                    sps = [ps.tile([128, 512], F32, tag="ps",
                                   name=f"s{blk}_{img}_{ich}_{jt}_{h}") for h in range(NH)]
                    for h in range(NH):
                        nc.tensor.matmul(
                            sps[h][:],
                            qk_img[32 * h:32 * h + 32, 1024 + jt * 128: 1024 + (jt + 1) * 128],
                            qk_img[32 * h:32 * h + 32, ich * 512: ich * 512 + 512],
                            start=True, stop=True, tile_position=(32 * h, 0))
                    pb = [tp4.tile([128, 512], BF16, tag="pb",
                                   name=f"p{blk}_{img}_{ich}_{jt}_{h}") for h in range(NH)]
                    for h in range(NH):
                        nc.scalar.activation(pb[h][:], sps[h][:], AF.Exp, scale=SCALE)
                    for h in range(NH):
                        nc.tensor.matmul(
                            pvs[h][0:65, :],
                            vt[:, jt * 260 + h * 65: jt * 260 + (h + 1) * 65],
                            pb[h][:], start=(jt == 0), stop=(jt == 7))
                for h in range(NH):
                    rs = tp.tile([1, 512], F32, tag="rs", name=f"rs{blk}_{img}_{ich}_{h}")
                    nc.scalar.copy(rs[:], pvs[h][64:65, :])
                    nc.vector.reciprocal(rs[:], rs[:])
                    rb = tp.tile([64, 512], F32, tag="rb", name=f"rb{blk}_{img}_{ich}_{h}")
                    nc.sync.dma_start(
                        rb[:], rs[:].unsqueeze(1).broadcast_to([1, 64, 512]))
                    ct, r0 = divmod(h * 64, 128)
                    nc.vector.tensor_tensor(
                        attn_f[r0:r0 + 64, ct * 1024 + ich * 512: ct * 1024 + ich * 512 + 512],
                        pvs[h][0:64, :], rb[:], ALU.mult)

            # dwconv 3x3 on raster v_sb -> pe (raster, f32), then permute+add
            pe = tp.tile([128, 2048], F32, tag="scr8", name=f"pe{blk}_{img}")
            for ct in range(2):
                out2 = pe[:, ct * 1024:(ct + 1) * 1024].rearrange(
                    "p (y x) -> p y x", y=32)
                in2 = v_sb[:, ct * 1024:(ct + 1) * 1024].rearrange(
                    "p (y x) -> p y x", y=32)
                # center tap first: full-coverage init (pure multiply)
                nc.vector.tensor_scalar(out2, in2, pew[:, ct * 9 + 4: ct * 9 + 5],
                                        None, ALU.mult)
                for tap in range(9):
                    if tap == 4:
                        continue
                    dy, dx = tap // 3 - 1, tap % 3 - 1
                    y0, y1 = max(0, -dy), min(32, 32 - dy)
                    x0, x1 = max(0, -dx), min(32, 32 - dx)
                    dst = out2[:, y0:y1, x0:x1]
                    src = in2[:, y0 + dy:y1 + dy, x0 + dx:x1 + dx]
                    nc.vector.scalar_tensor_tensor(
                        dst, src, pew[:, ct * 9 + tap: ct * 9 + tap + 1],
                        dst, ALU.mult, ALU.add)
            # pe (raster) -> wm-ordered bf16, then add into attn_f
            for ct in range(2):
                pewm = tp.tile([128, 1024], BF16, tag="pewm", name=f"pw{blk}_{img}_{ct}")
                src = pe[:, ct * 1024:(ct + 1) * 1024].rearrange(
                    "p (wy py wx px) -> p wy wx py px", wy=8, py=4, wx=8, px=4)
                dst = pewm[:].rearrange(
                    "p (wy wx py px) -> p wy wx py px", wy=8, wx=8, py=4, px=4)
                nc.vector.tensor_copy(dst, src)
                acol = ct * 1024
                nc.vector.tensor_tensor(attn_f[:, acol:acol + 1024],
                                        attn_f[:, acol:acol + 1024], pewm[:], ALU.add)

            attn_b = tp.tile([128, 2048], BF16, tag="attn_b", name=f"ab{blk}_{img}")
            nc.vector.tensor_copy(attn_b[:], attn_f[:])
            # proj + residual add into bfb
            for mt in range(2):
                for nch in range(2):
                    p = pst(f"projp{blk}_{img}_{mt}_{nch}")
                    for kt in range(2):
                        nc.tensor.matmul(
                            p[:], projw[:, kt * 256 + mt * 128: kt * 256 + (mt + 1) * 128],
                            attn_b[:, kt * 1024 + nch * 512: kt * 1024 + nch * 512 + 512],
                            start=(kt == 0), stop=(kt == 1))
                    dst = bfb[:, mt * NPIX + icol + nch * 512: mt * NPIX + icol + nch * 512 + 512]
                    nc.vector.tensor_tensor(dst, p[:], dst, ALU.add)

        # ---- fmffn ----
        # bfm <- bfb permuted to (wp, img, win) order, per ct
        for ct in range(2):
            for img in range(BPC):
                src = bfb[:, ct * NPIX + img * 1024: ct * NPIX + (img + 1) * 1024]
                src = src.rearrange("p (wy wx py px) -> p py px (wy wx)",
                                    wy=8, wx=8, py=4, px=4)
                dst = bfm[:, ct * NPIX:(ct + 1) * NPIX].rearrange(
                    "p (py px iw) -> p py px iw", py=4, px=4)[:, :, :, img * 64:(img + 1) * 64]
                nc.vector.tensor_copy(dst, src)
        t2_sb = big.tile([128, NKT2 * NPIX], BF16, tag="big", name=f"t2_{blk}")
        for mt in range(NKT2):
            m0 = mt * 128
            mrows = min(128, HID - m0)
            ngr = (mrows + 7) // 8
            wmt = tp.tile([128, 16 * 128], BF16, tag="wmt", name=f"wmt{blk}_{mt}")
            nc.sync.dma_start(
                wmt[:, 0:ngr * 128].rearrange("p (g m) -> p g m", g=ngr),
                d['wmb'][blk, mt * 16: mt * 16 + ngr].rearrange("g p m -> p g m"))
            t1c = tp.tile([128, NPIX], BF16, tag="scr8", name=f"t1c{blk}_{mt}")
            for ch in range(8):
                p = pst(f"fc1p{blk}_{mt}_{ch}")
                for kt in range(2):
                    nc.tensor.matmul(
                        p[0:mrows, :],
                        fc1w[:, kt * HID + m0: kt * HID + m0 + mrows],
                        bfm[:, kt * NPIX + ch * 512: kt * NPIX + (ch + 1) * 512],
                        start=(kt == 0), stop=(kt == 1))
                nc.scalar.activation(t1c[0:mrows, ch * 512:(ch + 1) * 512],
                                     p[0:mrows, :], AF.Gelu)
            for gl in range(ngr):
                g = mt * 16 + gl
                t1s = tp3.tile([128, NWIN], BF16, tag="t1s", name=f"t1s{blk}_{g}")
                nc.sync.dma_start(t1s[:], t1c[gl * 8:(gl + 1) * 8, :])
                p = ps.tile([128, 256], F32, tag="ps", name=f"winp{blk}_{g}")
                nc.tensor.matmul(p[:], wmt[:, gl * 128:(gl + 1) * 128], t1s[:],
                                 start=True, stop=True)
                t2s = tp3.tile([128, NWIN], BF16, tag="t2s", name=f"t2s{blk}_{g}")
                nc.scalar.copy(t2s[:], p[:])
                kt2, r0 = divmod(g * 8, 128)
                nc.sync.dma_start(t2_sb[r0:r0 + 8, kt2 * NPIX:(kt2 + 1) * NPIX], t2s[:])
        # fc2 + residual
        for img in range(BPC):
            for mt in range(2):
                for nch in range(2):
                    p = pst(f"fc2p{blk}_{img}_{mt}_{nch}")
                    for kt in range(NKT2):
                        krows = min(128, HID - kt * 128)
                        rhs = t2_sb[0:krows, kt * NPIX:(kt + 1) * NPIX].rearrange(
                            "p (wp iw) -> p wp iw", wp=16)[:, :, img * 64:(img + 1) * 64]
                        rhs = rhs.transpose([0, 2, 1])[:, nch * 32:(nch + 1) * 32, :]
                        nc.tensor.matmul(
                            p[:], fc2w[0:krows, kt * 256 + mt * 128: kt * 256 + (mt + 1) * 128],
                            rhs, start=(kt == 0), stop=(kt == NKT2 - 1))
                    col = img * 1024 + nch * 512
                    dst = bfb[:, mt * NPIX + col: mt * NPIX + col + 512]
                    nc.vector.tensor_tensor(dst, p[:], dst, ALU.add)

    # ---------------- cv2 + silu -> y (raster) ----------------
    for mt in range(4):
        for img in range(BPC):
            for nch in range(2):
                p = pst(f"cv2p_{mt}_{img}_{nch}")
                col = img * 1024 + nch * 512
                for kt in range(4):
                    rhs = (a_sb if kt < 2 else bfb)[
                        :, (kt % 2) * NPIX + col: (kt % 2) * NPIX + col + 512]
                    nc.tensor.matmul(
                        p[:], cv2w[:, kt * 512 + mt * 128: kt * 512 + (mt + 1) * 128],
                        rhs, start=(kt == 0), stop=(kt == 3))
                yc = tp.tile([128, 512], F32, tag="yc", name=f"yc{mt}_{img}_{nch}")
                nc.scalar.activation(yc[:], p[:], AF.Silu)
                # yc cols (wy4, wx8, py4, px4) window-major -> raster in yr
                yr = tp.tile([128, 512], F32, tag="yr", name=f"yr{mt}_{img}_{nch}")
                dst = yr[:].rearrange("p (wy py wx px) -> p wy wx py px",
                                      wy=4, py=4, wx=8, px=4)
                nc.vector.tensor_copy(
                    dst, yc[:].rearrange("p (wy wx py px) -> p wy wx py px",
                                         wy=4, wx=8, py=4, px=4))
                nc.sync.dma_start(
                    d['y'][mt * 128:(mt + 1) * 128, col:col + 512], yr[:])


_BUILT = {}


def _build():
    import os as _os
    _key = 'nc' + _os.environ.get("KREP", "1")
    if _key in _BUILT:
        return _BUILT[_key]
    nc = bacc.Bacc("TRN2", target_bir_lowering=False, debug=False, num_devices=NCORE)
    d = {
        'x': nc.dram_tensor("x", [C1, NPIX], F32, kind="ExternalInput").ap(),
        'cv1_wT': nc.dram_tensor("cv1_wT", [C1, C1], BF16, kind="ExternalInput").ap(),
        'cv2_wT': nc.dram_tensor("cv2_wT", [C1, C1], BF16, kind="ExternalInput").ap(),
        'qk_wT': nc.dram_tensor("qk_wT", [NB, C, C], BF16, kind="ExternalInput").ap(),
        'v_wT': nc.dram_tensor("v_wT", [NB, C, C], BF16, kind="ExternalInput").ap(),
        'proj_wT': nc.dram_tensor("proj_wT", [NB, C, C], BF16, kind="ExternalInput").ap(),
        'pe_w': nc.dram_tensor("pe_w", [NB, C, 9], F32, kind="ExternalInput").ap(),
        'fc1_wT': nc.dram_tensor("fc1_wT", [NB, C, HID], BF16, kind="ExternalInput").ap(),
        'fc2_wT': nc.dram_tensor("fc2_wT", [NB, HID, C], BF16, kind="ExternalInput").ap(),
        'wmb': nc.dram_tensor("wmb", [NB, NG, 128, 128], BF16, kind="ExternalInput").ap(),
        'y': nc.dram_tensor("y", [C1, NPIX], F32, kind="ExternalOutput").ap(),
    }
    with tile.TileContext(nc) as tc:
        with ExitStack() as ctx:
            _emit(ctx, nc, tc, d)
    nc.compile()
    _BUILT[_key] = nc
    return nc


def kernel(**inputs):
    nc = _build()
    w = _prep_weights(inputs)
    x = np.asarray(inputs['x'], np.float32)
    in_maps = []
    for core in range(NCORE):
        xs = np.ascontiguousarray(
            x[core * BPC:(core + 1) * BPC].transpose(1, 0, 2, 3).reshape(C1, NPIX))
        m = {'x': xs}
        m.update(w)
        in_maps.append(m)
    res = bass_utils.run_bass_kernel_spmd(nc, in_maps, core_ids=list(range(NCORE)))
    outs = []
    for core in range(NCORE):
        y = res.results[core]['y']                      # [512, NPIX] raster
        outs.append(y.reshape(C1, BPC, 32, 32).transpose(1, 0, 2, 3))
    return np.concatenate(outs, 0)


# revision 20
# speedup vs baseline: 1.2592x; 1.1600x over previous
"""Trainium2 Bass kernel for nn_C2PSA_FMFFN (C2PSA with frequency-modulated FFN).

Sharding: data-parallel over batch, B=32 -> 4 images per core on 8 cores.

Per-core layout: activations as [channels(partitions), pixels(free)], pixels in
window-major order (img, wy, wx, py, px).  Restructurings:
 - FMFFN's rfft2 -> complex modulation -> irfft2 == per-channel 4x4 circular
   convolution within each 4x4 window == per-channel 16x16 matrix; executed on
   the tensor engine as 85 block-diagonal [128x128] matmuls (8 channels x 16
   window-pixels) with SBUF<->SBUF shuffle DMAs around it.
 - Attention computed as S^T = k^T q (4 heads concurrently via PE row tiling),
   softmax without max-subtraction (scores are small; exp stays in fp32 range),
   denominators from an appended ones-column on v^T inside the PV matmul.
 - Depthwise 3x3 conv as 9 per-partition-scalar multiply-accumulate DVE ops
   decomposed into affine pieces of the window-major layout.
GEMMs in bf16 with fp32 PSUM accumulation; softmax/normalization in fp32.
"""
import numpy as np
import ml_dtypes
from contextlib import ExitStack

import concourse.bass as bass
import concourse.tile as tile
from concourse import bacc, mybir
from concourse import bass_utils

F32, BF16 = mybir.dt.float32, mybir.dt.bfloat16
AF = mybir.ActivationFunctionType
ALU = mybir.AluOpType

B, C1, C, NH, KD, HD, HID, NB, WS = 32, 512, 256, 4, 32, 64, 680, 3, 4
SCALE = KD ** -0.5
NCORE = 8
BPC = B // NCORE              # images per core = 4
NPIX = BPC * 1024             # 4096
NWIN = BPC * 64               # 256 windows per core
NG = HID // 8                 # 85 window channel-groups
NKT2 = (HID + 127) // 128     # 6 k-tiles over HID

bf16 = ml_dtypes.bfloat16


# ------------------------------------------------------------------ host prep
def _win_kernels(cw_i):
    """cw_i: [4, 3, HID, 2] -> real circular conv kernels [HID, 4, 4]."""
    wc = cw_i[..., 0] + 1j * cw_i[..., 1]
    delta = np.zeros((4, 4, 1))
    delta[0, 0, 0] = 1.0
    f = np.fft.rfft2(delta, axes=(0, 1), norm='ortho')
    h = np.fft.irfft2(f * wc, s=(4, 4), axes=(0, 1), norm='ortho')
    return np.transpose(h, (2, 0, 1))


def _win_blockdiag(cw):
    """cw: [NB,4,3,HID,2] -> [NB,NG,128,128] block-diag lhsT (bf16).
    lhsT[(c8,q),(c8,p)] = M_c[p,q];  M_c[p,q] = h_c[(py-qy)%4,(px-qx)%4]."""
    out = np.zeros((NB, NG, 128, 128), np.float32)
    for i in range(NB):
        h = _win_kernels(cw[i])                       # [HID,4,4]
        M = np.zeros((HID, 16, 16), np.float32)
        for pp in range(16):
            ppy, ppx = divmod(pp, 4)
            for qq in range(16):
                qqy, qqx = divmod(qq, 4)
                M[:, pp, qq] = h[:, (ppy - qqy) % 4, (ppx - qqx) % 4]
        for g in range(NG):
            for c8 in range(8):
                out[i, g, c8 * 16:(c8 + 1) * 16, c8 * 16:(c8 + 1) * 16] = M[g * 8 + c8].T
    return out.astype(bf16)


def _prep_weights(inp):
    qkv = np.asarray(inp['qkv_w'], np.float32).reshape(NB, NH, 128, C)
    wq = qkv[:, :, :32].reshape(NB, NH * 32, C)
    wk = qkv[:, :, 32:64].reshape(NB, NH * 32, C)
    wv = qkv[:, :, 64:].reshape(NB, NH * 64, C)
    return {
        'cv1_wT': np.asarray(inp['cv1_w'], np.float32).T.astype(bf16).copy(),
        'cv2_wT': np.asarray(inp['cv2_w'], np.float32).T.astype(bf16).copy(),
        'qk_wT': np.concatenate([wq, wk], 1).transpose(0, 2, 1).astype(bf16).copy(),
        'v_wT': wv.transpose(0, 2, 1).astype(bf16).copy(),
        'proj_wT': np.asarray(inp['proj_w'], np.float32).transpose(0, 2, 1).astype(bf16).copy(),
        'pe_w': np.asarray(inp['pe_w'], np.float32).reshape(NB, C, 9).copy(),
        'fc1_wT': np.asarray(inp['fc1_w'], np.float32).transpose(0, 2, 1).astype(bf16).copy(),
        'fc2_wT': np.asarray(inp['fc2_w'], np.float32).transpose(0, 2, 1).astype(bf16).copy(),
        'wmb': _win_blockdiag(np.asarray(inp['cw'], np.float64)),
    }


def _dpieces(dd):
    """(d0, nd, pd0, npd, sw, sp): dst (w, p) ranges + src offsets for shift dd."""
    if dd == 0:
        return [(0, 8, 0, 4, 0, 0)]
    if dd == 1:
        return [(0, 8, 0, 3, 0, 1), (0, 7, 3, 1, 1, -3)]
    return [(0, 8, 1, 3, 0, -1), (1, 7, 0, 1, -1, 3)]


# ------------------------------------------------------------------ emit
def _emit(ctx, nc, tc, d):
    sb = ctx.enter_context(tc.tile_pool(name="sb", bufs=1))
    ps = ctx.enter_context(tc.tile_pool(name="ps", bufs=8, space="PSUM"))
    wpool = ctx.enter_context(tc.tile_pool(name="wpool", bufs=1))
    big = ctx.enter_context(tc.tile_pool(name="big", bufs=1))
    tp = ctx.enter_context(tc.tile_pool(name="tp", bufs=2))
    tp3 = ctx.enter_context(tc.tile_pool(name="tp3", bufs=3))
    tp4 = ctx.enter_context(tc.tile_pool(name="tp4", bufs=4))
    tp8 = ctx.enter_context(tc.tile_pool(name="tp8", bufs=8))

    def pst(name):
        return ps.tile([128, 512], F32, tag="ps", name=name)

    def load_w(name, dram, ktiles, mcols, dtype=BF16):
        t = wpool.tile([128, ktiles * mcols], dtype, tag=name, name=name)
        nc.sync.dma_start(t[:].rearrange("p (k m) -> p k m", k=ktiles),
                          dram.rearrange("(k p) m -> p k m", p=128))
        return t

    import os as _os
    for _rep in range(int(_os.environ.get("KREP", "1"))):
        _emit_body(nc, tc, d, pst, load_w, sb, ps, wpool, big, tp, tp3, tp4, tp8)


def _emit_body(nc, tc, d, pst, load_w, sb, ps, wpool, big, tp, tp3, tp4, tp8):
    # ---------------- static weights ----------------
    cv1w = load_w("cv1w", d['cv1_wT'][:], 4, 512)
    cv2w = load_w("cv2w", d['cv2_wT'][:], 4, 512)

    # ---------------- input -> xb bf16 window-major ----------------
    xb = big.tile([128, 4 * NPIX], BF16, tag="big", name="xb")
    for kt in range(4):
        for img in range(BPC):
            for hh in range(2):
                xc = tp.tile([128, 512], F32, tag="xc", name=f"xc{kt}_{img}_{hh}")
                nc.sync.dma_start(
                    xc[:], d['x'][kt * 128:(kt + 1) * 128,
                                  img * 1024 + hh * 512: img * 1024 + (hh + 1) * 512])
                src = xc[:].rearrange("p (wy py wx px) -> p wy py wx px",
                                      wy=4, py=4, wx=8, px=4).transpose([0, 1, 3, 2, 4])
                dst = xb[:, kt * NPIX + img * 1024 + hh * 512:
                         kt * NPIX + img * 1024 + (hh + 1) * 512]
                dst = dst.rearrange("p (wy wx py px) -> p wy wx py px",
                                    wy=4, wx=8, py=4, px=4)
                nc.vector.tensor_copy(dst, src)

    # ---------------- cv1 + silu ----------------
    a_sb = sb.tile([128, 2 * NPIX], BF16, name="a_sb")
    bfb = sb.tile([128, 2 * NPIX], BF16, name="bfb")
    for mt in range(4):
        for img in range(BPC):
            for nch in range(2):
                p = pst(f"cv1p_{mt}_{img}_{nch}")
                col = img * 1024 + nch * 512
                for kt in range(4):
                    nc.tensor.matmul(
                        p[:], cv1w[:, kt * 512 + mt * 128: kt * 512 + (mt + 1) * 128],
                        xb[:, kt * NPIX + col: kt * NPIX + col + 512],
                        start=(kt == 0), stop=(kt == 3))
                dst = (a_sb if mt < 2 else bfb)
                mm = mt % 2
                nc.scalar.activation(dst[:, mm * NPIX + col: mm * NPIX + col + 512],
                                     p[:], AF.Silu)

    # ---------------- blocks ----------------
    bfm = sb.tile([128, 2 * NPIX], BF16, name="bfm")       # (wp, img, win) order
    for blk in range(NB):
        qkw = load_w(f"qkw", d['qk_wT'][blk], 2, 256)
        vw = load_w(f"vw", d['v_wT'][blk], 2, 256)
        projw = load_w(f"projw", d['proj_wT'][blk], 2, 256)
        pew = load_w(f"pew", d['pe_w'][blk], 2, 9, dtype=F32)
        fc1w = load_w(f"fc1w", d['fc1_wT'][blk], 2, HID)
        fc2w = wpool.tile([128, NKT2 * 256], BF16, tag="fc2w", name=f"fc2w{blk}")
        nc.sync.dma_start(
            fc2w[:].rearrange("p (k m) -> p k m", k=NKT2)[:, 0:5],
            d['fc2_wT'][blk, 0:640].rearrange("(k p) m -> p k m", p=128))
        nc.sync.dma_start(fc2w[0:40, 5 * 256:6 * 256], d['fc2_wT'][blk, 640:680, :])

        # ---- attention, per image ----
        for img in range(BPC):
            icol = img * 1024
            # qk GEMM -> qk_img [128, 2048] (cols: q 0-1023, k 1024-2047)
            qk_img = tp.tile([128, 2048], BF16, tag="qk", name=f"qk{blk}_{img}")
            for mt in range(2):
                for nch in range(2):
                    p = pst(f"qkp{blk}_{img}_{mt}_{nch}")
                    for kt in range(2):
                        nc.tensor.matmul(
                            p[:], qkw[:, kt * 256 + mt * 128: kt * 256 + (mt + 1) * 128],
                            bfb[:, kt * NPIX + icol + nch * 512: kt * NPIX + icol + nch * 512 + 512],
                            start=(kt == 0), stop=(kt == 1))
                    nc.vector.tensor_copy(
                        qk_img[:, mt * 1024 + nch * 512: mt * 1024 + nch * 512 + 512], p[:])

            # vT GEMM -> vt [128, 8*260] (j-tile major; cols h*65+d, col 64 = ones)
            vt = tp.tile([128, 8 * 512], BF16, tag="vt", name=f"vt{blk}_{img}")
            for jt in range(8):
                p = pst(f"vtp{blk}_{img}_{jt}")
                for kt in range(2):
                    nc.tensor.matmul(
                        p[:, 0:256],
                        bfb[:, kt * NPIX + icol + jt * 128: kt * NPIX + icol + (jt + 1) * 128],
                        vw[:, kt * 256:(kt + 1) * 256],
                        start=(kt == 0), stop=(kt == 1))
                dst = vt[:, jt * 512:(jt + 1) * 512].rearrange("p (h e) -> p h e", h=4)
                nc.vector.tensor_copy(dst[:, :, 0:64],
                                      p[:, 0:256].rearrange("p (h e) -> p h e", h=4))
                nc.vector.memset(dst[:, :, 64:128], 1.0)

            # v GEMM -> v_sb [256ch, 1024] bf16 in RASTER order (dwconv input)
            v_sb = tp.tile([128, 2 * 1024], BF16, tag="v_sb", name=f"v{blk}_{img}")
            for ct in range(2):
                for nch in range(2):
                    p = pst(f"vp{blk}_{img}_{ct}_{nch}")
                    for kt in range(2):
                        nc.tensor.matmul(
                            p[:], vw[:, kt * 256 + ct * 128: kt * 256 + (ct + 1) * 128],
                            bfb[:, kt * NPIX + icol + nch * 512: kt * NPIX + icol + nch * 512 + 512],
                            start=(kt == 0), stop=(kt == 1))
                    # psum cols (wy4, wx8, py4, px4) wm -> raster dst (4D copy)
                    dst = v_sb[:, ct * 1024 + nch * 512: ct * 1024 + nch * 512 + 512]
                    dst = dst.rearrange("p (wy py wx px) -> p wy wx py px",
                                        wy=4, py=4, wx=8, px=4)
                    nc.vector.tensor_copy(dst, p[:])

            # S^T + PV per i-chunk
            attn_f = tp.tile([128, 2048], F32, tag="scr8", name=f"at{blk}_{img}")
            for ich in range(2):
                pvs = [ps.tile([128, 512], F32, tag="ps", name=f"pv{blk}_{img}_{ich}_{h}")
                       for h in range(NH)]
                for jt in range(8):
                    sps = [ps.tile([128, 512], F32, tag="ps",
                                   name=f"s{blk}_{img}_{ich}_{jt}_{h}") for h in range(NH)]
                    for h in range(NH):
                        nc.tensor.matmul(
                            sps[h][:],
                            qk_img[32 * h:32 * h + 32, 1024 + jt * 128: 1024 + (jt + 1) * 128],
                            qk_img[32 * h:32 * h + 32, ich * 512: ich * 512 + 512],
                            start=True, stop=True, tile_position=(32 * h, 0))
                    pb = [tp8.tile([128, 512], BF16, tag="pb",
                                   name=f"p{blk}_{img}_{ich}_{jt}_{h}") for h in range(NH)]
                    for h in range(NH):
                        nc.scalar.activation(pb[h][:], sps[h][:], AF.Exp, scale=SCALE)
                    for h in range(NH):
                        nc.tensor.matmul(
                            pvs[h][:],
                            vt[:, jt * 512 + h * 128: jt * 512 + (h + 1) * 128],
                            pb[h][:], start=(jt == 0), stop=(jt == 7))
                for h in range(NH):
                    rb = tp.tile([64, 512], F32, tag="rb", name=f"rb{blk}_{img}_{ich}_{h}")
                    nc.vector.reciprocal(rb[:], pvs[h][64:128, :])
                    ct, r0 = divmod(h * 64, 128)
                    nc.vector.tensor_tensor(
                        attn_f[r0:r0 + 64, ct * 1024 + ich * 512: ct * 1024 + ich * 512 + 512],
                        pvs[h][0:64, :], rb[:], ALU.mult)

            # dwconv 3x3 on raster v_sb -> pe (raster, f32), then permute+add
            pe = tp.tile([128, 2048], F32, tag="scr8", name=f"pe{blk}_{img}")
            for ct in range(2):
                out2 = pe[:, ct * 1024:(ct + 1) * 1024].rearrange(
                    "p (y x) -> p y x", y=32)
                in2 = v_sb[:, ct * 1024:(ct + 1) * 1024].rearrange(
                    "p (y x) -> p y x", y=32)
                # center tap first: full-coverage init (pure multiply)
                nc.vector.tensor_scalar(out2, in2, pew[:, ct * 9 + 4: ct * 9 + 5],
                                        None, ALU.mult)
                for tap in range(9):
                    if tap == 4:
                        continue
                    dy, dx = tap // 3 - 1, tap % 3 - 1
                    y0, y1 = max(0, -dy), min(32, 32 - dy)
                    x0, x1 = max(0, -dx), min(32, 32 - dx)
                    dst = out2[:, y0:y1, x0:x1]
                    src = in2[:, y0 + dy:y1 + dy, x0 + dx:x1 + dx]
                    nc.vector.scalar_tensor_tensor(
                        dst, src, pew[:, ct * 9 + tap: ct * 9 + tap + 1],
                        dst, ALU.mult, ALU.add)
            # pe (raster) -> wm-ordered bf16, then add into attn_f
            for ct in range(2):
                pewm = tp.tile([128, 1024], BF16, tag="pewm", name=f"pw{blk}_{img}_{ct}")
                src = pe[:, ct * 1024:(ct + 1) * 1024].rearrange(
                    "p (wy py wx px) -> p wy wx py px", wy=8, py=4, wx=8, px=4)
                dst = pewm[:].rearrange(
                    "p (wy wx py px) -> p wy wx py px", wy=8, wx=8, py=4, px=4)
                nc.vector.tensor_copy(dst, src)
                acol = ct * 1024
                nc.vector.tensor_tensor(attn_f[:, acol:acol + 1024],
                                        attn_f[:, acol:acol + 1024], pewm[:], ALU.add)

            attn_b = tp.tile([128, 2048], BF16, tag="attn_b", name=f"ab{blk}_{img}")
            nc.vector.tensor_copy(attn_b[:], attn_f[:])
            # proj + residual add into bfb
            for mt in range(2):
                for nch in range(2):
                    p = pst(f"projp{blk}_{img}_{mt}_{nch}")
                    for kt in range(2):
                        nc.tensor.matmul(
                            p[:], projw[:, kt * 256 + mt * 128: kt * 256 + (mt + 1) * 128],
                            attn_b[:, kt * 1024 + nch * 512: kt * 1024 + nch * 512 + 512],
                            start=(kt == 0), stop=(kt == 1))
                    dst = bfb[:, mt * NPIX + icol + nch * 512: mt * NPIX + icol + nch * 512 + 512]
                    nc.vector.tensor_tensor(dst, p[:], dst, ALU.add)

        # ---- fmffn ----
        # bfm <- bfb permuted to (wp, img, win) order, per ct
        for ct in range(2):
            for img in range(BPC):
                src = bfb[:, ct * NPIX + img * 1024: ct * NPIX + (img + 1) * 1024]
                src = src.rearrange("p (wy wx py px) -> p py px (wy wx)",
                                    wy=8, wx=8, py=4, px=4)
                dst = bfm[:, ct * NPIX:(ct + 1) * NPIX].rearrange(
                    "p (py px iw) -> p py px iw", py=4, px=4)[:, :, :, img * 64:(img + 1) * 64]
                nc.vector.tensor_copy(dst, src)
        t2_sb = big.tile([128, NKT2 * NPIX], BF16, tag="big", name=f"t2_{blk}")
        for mt in range(NKT2):
            m0 = mt * 128
            mrows = min(128, HID - m0)
            ngr = (mrows + 7) // 8
            wmt = wpool.tile([128, 16 * 128], BF16, tag="wmt", name=f"wmt{blk}_{mt}")
            nc.sync.dma_start(
                wmt[:, 0:ngr * 128].rearrange("p (g m) -> p g m", g=ngr),
                d['wmb'][blk, mt * 16: mt * 16 + ngr].rearrange("g p m -> p g m"))
            t1c = tp.tile([128, NPIX], BF16, tag="scr8", name=f"t1c{blk}_{mt}")
            for ch in range(8):
                p = pst(f"fc1p{blk}_{mt}_{ch}")
                for kt in range(2):
                    nc.tensor.matmul(
                        p[0:mrows, :],
                        fc1w[:, kt * HID + m0: kt * HID + m0 + mrows],
                        bfm[:, kt * NPIX + ch * 512: kt * NPIX + (ch + 1) * 512],
                        start=(kt == 0), stop=(kt == 1))
                nc.scalar.activation(t1c[0:mrows, ch * 512:(ch + 1) * 512],
                                     p[0:mrows, :], AF.Gelu)
            dma_engs = [nc.gpsimd, nc.sync, nc.scalar]
            for gl in range(ngr):
                g = mt * 16 + gl
                t1s = tp3.tile([128, NWIN], BF16, tag="t1s", name=f"t1s{blk}_{g}")
                dma_engs[gl % 3].dma_start(t1s[:], t1c[gl * 8:(gl + 1) * 8, :])
                p = ps.tile([128, 256], F32, tag="ps", name=f"winp{blk}_{g}")
                nc.tensor.matmul(p[:], wmt[:, gl * 128:(gl + 1) * 128], t1s[:],
                                 start=True, stop=True)
                t2s = tp3.tile([128, NWIN], BF16, tag="t2s", name=f"t2s{blk}_{g}")
                nc.vector.tensor_copy(t2s[:], p[:])
                kt2, r0 = divmod(g * 8, 128)
                dma_engs[(gl + 1) % 3].dma_start(
                    t2_sb[r0:r0 + 8, kt2 * NPIX:(kt2 + 1) * NPIX], t2s[:])
        # fc2 + residual
        for img in range(BPC):
            for mt in range(2):
                for nch in range(2):
                    p = pst(f"fc2p{blk}_{img}_{mt}_{nch}")
                    for kt in range(NKT2):
                        krows = min(128, HID - kt * 128)
                        rhs = t2_sb[0:krows, kt * NPIX:(kt + 1) * NPIX].rearrange(
                            "p (wp iw) -> p wp iw", wp=16)[:, :, img * 64:(img + 1) * 64]
                        rhs = rhs.transpose([0, 2, 1])[:, nch * 32:(nch + 1) * 32, :]
                        nc.tensor.matmul(
                            p[:], fc2w[0:krows, kt * 256 + mt * 128: kt * 256 + (mt + 1) * 128],
                            rhs, start=(kt == 0), stop=(kt == NKT2 - 1))
                    col = img * 1024 + nch * 512
                    dst = bfb[:, mt * NPIX + col: mt * NPIX + col + 512]
                    nc.vector.tensor_tensor(dst, p[:], dst, ALU.add)

    # ---------------- cv2 + silu -> y (raster) ----------------
    for mt in range(4):
        for img in range(BPC):
            for nch in range(2):
                p = pst(f"cv2p_{mt}_{img}_{nch}")
                col = img * 1024 + nch * 512
                for kt in range(4):
                    rhs = (a_sb if kt < 2 else bfb)[
                        :, (kt % 2) * NPIX + col: (kt % 2) * NPIX + col + 512]
                    nc.tensor.matmul(
                        p[:], cv2w[:, kt * 512 + mt * 128: kt * 512 + (mt + 1) * 128],
                        rhs, start=(kt == 0), stop=(kt == 3))
                yc = tp.tile([128, 512], F32, tag="yc", name=f"yc{mt}_{img}_{nch}")
                nc.scalar.activation(yc[:], p[:], AF.Silu)
                # yc cols (wy4, wx8, py4, px4) window-major -> raster in yr
                yr = tp.tile([128, 512], F32, tag="yr", name=f"yr{mt}_{img}_{nch}")
                dst = yr[:].rearrange("p (wy py wx px) -> p wy wx py px",
                                      wy=4, py=4, wx=8, px=4)
                nc.vector.tensor_copy(
                    dst, yc[:].rearrange("p (wy wx py px) -> p wy wx py px",
                                         wy=4, wx=8, py=4, px=4))
                nc.sync.dma_start(
                    d['y'][mt * 128:(mt + 1) * 128, col:col + 512], yr[:])


_BUILT = {}


def _build():
    import os as _os
    _key = 'nc' + _os.environ.get("KREP", "1")
    if _key in _BUILT:
        return _BUILT[_key]
    nc = bacc.Bacc("TRN2", target_bir_lowering=False, debug=False, num_devices=NCORE)
    d = {
        'x': nc.dram_tensor("x", [C1, NPIX], F32, kind="ExternalInput").ap(),
        'cv1_wT': nc.dram_tensor("cv1_wT", [C1, C1], BF16, kind="ExternalInput").ap(),
        'cv2_wT': nc.dram_tensor("cv2_wT", [C1, C1], BF16, kind="ExternalInput").ap(),
        'qk_wT': nc.dram_tensor("qk_wT", [NB, C, C], BF16, kind="ExternalInput").ap(),
        'v_wT': nc.dram_tensor("v_wT", [NB, C, C], BF16, kind="ExternalInput").ap(),
        'proj_wT': nc.dram_tensor("proj_wT", [NB, C, C], BF16, kind="ExternalInput").ap(),
        'pe_w': nc.dram_tensor("pe_w", [NB, C, 9], F32, kind="ExternalInput").ap(),
        'fc1_wT': nc.dram_tensor("fc1_wT", [NB, C, HID], BF16, kind="ExternalInput").ap(),
        'fc2_wT': nc.dram_tensor("fc2_wT", [NB, HID, C], BF16, kind="ExternalInput").ap(),
        'wmb': nc.dram_tensor("wmb", [NB, NG, 128, 128], BF16, kind="ExternalInput").ap(),
        'y': nc.dram_tensor("y", [C1, NPIX], F32, kind="ExternalOutput").ap(),
    }
    with tile.TileContext(nc) as tc:
        with ExitStack() as ctx:
            _emit(ctx, nc, tc, d)
    nc.compile()
    _BUILT[_key] = nc
    return nc


def kernel(**inputs):
    nc = _build()
    w = _prep_weights(inputs)
    x = np.asarray(inputs['x'], np.float32)
    in_maps = []
    for core in range(NCORE):
        xs = np.ascontiguousarray(
            x[core * BPC:(core + 1) * BPC].transpose(1, 0, 2, 3).reshape(C1, NPIX))
        m = {'x': xs}
        m.update(w)
        in_maps.append(m)
    res = bass_utils.run_bass_kernel_spmd(nc, in_maps, core_ids=list(range(NCORE)))
    outs = []
    for core in range(NCORE):
        y = res.results[core]['y']                      # [512, NPIX] raster
        outs.append(y.reshape(C1, BPC, 32, 32).transpose(1, 0, 2, 3))
    return np.concatenate(outs, 0)
